# revision 1
# baseline (speedup 1.0000x reference)
"""Trainium2 Bass kernel for topk_masking:  out = X + alpha * (top32_mask(A) @ X).

Row-parallel across 8 NeuronCores (A sharded [1024, 8192] per core, X
replicated).  Per 128-row batch on each core, split into quarter-row tiles
(1 MB loads) for fine-grained pipelining:
  * VectorE: per-segment max8 over each quarter -> candidate top-8s, then 4
    rounds of max+match_replace over the candidates -> top-32 values;
    t32 = 32nd largest.  Exact unless >8 of a row's top-32 fall in one
    segment (9 rows for this data; detected and host-fixed).
  * ScalarE: maskpm = Sign(A - prevfloat(t32)) in bf16 (+1 selected, -1 not),
    with fused accumulation as an exactness detector (catches segment
    overflow, boundary-value ties, Sign==0).
  * GPSIMD dma_gather(transpose): maskpm half -> maskT chunks [128j, 128row].
  * TensorE: psum = maskpm @ Xb (64 accumulated chunk matmuls, bf16).
    mask01 @ X = (maskpm @ X + colsum(X)) / 2, so
    out = [X_self + (a/2) colsum] + (a/2) psum, with colsum(Xb) from an
    all-ones matmul once per core.
  * VectorE: out = (a/2) * psum + Xmod; DMA out.
Host: rows whose detector count != 32 are recomputed exactly (~11 rows).
"""

import os
import numpy as np

N = 8192
HALF = N // 2
D = 256
K = 32
NCORES = 8
RPC = N // NCORES          # rows per core = 1024
BATCH = 128
NBATCH = RPC // BATCH      # 8
SEG = int(os.environ.get("TOPK_SEG", "512"))
NCH = N // 128             # 64 contraction chunks
HCH = NCH // 2
NEG_BIG = -1e30
ONE_MINUS_EPS = float(np.float32(1.0) - np.float32(2.0 ** -24))

last_results = None
_nc_cache = {}


def _build_cached(loop_reps=1, seg=None):
    key = (loop_reps, seg or SEG)
    if key not in _nc_cache:
        _nc_cache[key] = _build(loop_reps, seg)
    return _nc_cache[key]


def _build(loop_reps=1, seg=None):
    import concourse.bacc as bacc
    import concourse.mybir as mybir
    from concourse.tile import TileContext
    from concourse import library_config

    seg = seg or SEG
    nseg = N // seg            # segments per full row
    hseg = nseg // 2           # segments per half
    fp32 = mybir.dt.float32
    bf16 = mybir.dt.bfloat16
    add = mybir.AluOpType.add
    mult = mybir.AluOpType.mult
    Sign = mybir.ActivationFunctionType.Sign
    Copy = mybir.ActivationFunctionType.Copy

    nc = bacc.Bacc("TRN2", debug=False)
    a_in = nc.declare_dram_parameter("a", [RPC, N], fp32, isOutput=False)
    xb_in = nc.declare_dram_parameter("xb", [N, D], bf16, isOutput=False)
    xs_in = nc.declare_dram_parameter("xself", [RPC, D], fp32, isOutput=False)
    al_in = nc.declare_dram_parameter("alpha_h", [128, 1], fp32, isOutput=False)
    ti_in = nc.declare_dram_parameter("tidx", [128, 8], mybir.dt.int16, isOutput=False)
    out_ext = nc.declare_dram_parameter("out", [RPC, D], fp32, isOutput=True)
    cnt_ext = nc.declare_dram_parameter("count", [RPC, 1], fp32, isOutput=True)

    abufs = int(os.environ.get("TOPK_ABUFS", "3"))

    with TileContext(nc) as tc:
        with (
            tc.tile_pool(name="persist", bufs=1) as persist,
            tc.tile_pool(name="apool", bufs=abufs) as apool,
            tc.tile_pool(name="mpool", bufs=2) as mpool,
            tc.tile_pool(name="mtpool", bufs=int(os.environ.get("TOPK_MTBUFS", "2"))) as mtpool,
            tc.tile_pool(name="small", bufs=2) as small,
            tc.tile_pool(name="psum", bufs=2, space="PSUM") as psum_pool,
            tc.tile_pool(name="psumc", bufs=1, space="PSUM") as psumc_pool,
        ):
            nc.gpsimd.load_library(library_config.mlp)

            at_tiles = {}

            QW4 = N // 4

            def load_at(b):
                qs = []
                for q in range(4):
                    atq = apool.tile([128, QW4], fp32, tag=f"at{q}")
                    nc.sync.dma_start(
                        out=atq[:],
                        in_=a_in[b * BATCH:(b + 1) * BATCH, q * QW4:(q + 1) * QW4])
                    qs.append(atq)
                at_tiles[b] = qs

            if loop_reps == 1:
                load_at(0)
                load_at(1)

            tidx = persist.tile([128, 8], mybir.dt.int16)
            nc.scalar.dma_start(out=tidx[:], in_=ti_in[:])

            # X resident in bf16, chunk-major: xb[p, c*D + d] = X[c*128 + p, d]
            xb = persist.tile([128, NCH * D], bf16)
            nc.scalar.dma_start(
                out=xb[:].rearrange("p (c d) -> p c d", d=D),
                in_=xb_in.rearrange("(c p) d -> p c d", p=128),
            )
            alpha_h = persist.tile([128, 1], fp32)
            nc.scalar.dma_start(out=alpha_h[:], in_=al_in[:])
            cnt_all = persist.tile([128, NBATCH], fp32)

            xv = xb[:].rearrange("p (c d) -> p c d", d=D)

            # colsum(Xb) broadcast to 128 rows via all-ones matmul
            ones_sb = persist.tile([128, 128], bf16)
            nc.vector.memset(ones_sb[:], 1.0)
            ps_cs = psumc_pool.tile([128, D], fp32)
            for c in range(NCH):
                nc.tensor.matmul(ps_cs[:], lhsT=ones_sb[:], rhs=xv[:, c, :],
                                 start=(c == 0), stop=(c == NCH - 1))

            # Xmod = X_self + (alpha/2) * colsum   (per 128-row slice)
            xmod = persist.tile([128, NBATCH * D], fp32)
            for b in range(NBATCH):
                xs = small.tile([128, D], fp32)
                nc.scalar.dma_start(out=xs[:], in_=xs_in[b * BATCH:(b + 1) * BATCH, :])
                nc.vector.scalar_tensor_tensor(
                    out=xmod[:, b * D:(b + 1) * D], in0=ps_cs[:],
                    scalar=alpha_h[:, 0:1], in1=xs[:], op0=mult, op1=add)

            def batch_body(b):
                if b + 2 < NBATCH:
                    load_at(b + 2)
                atq = at_tiles.pop(b)

                # per-segment top-8 candidates, quarter by quarter
                qseg = QW4 // seg
                cands = small.tile([128, nseg * 8], fp32)
                for q in range(4):
                    for s in range(qseg):
                        g = q * qseg + s
                        nc.vector.max(out=cands[:, g * 8:(g + 1) * 8],
                                      in_=atq[q][:, s * seg:(s + 1) * seg])

                # top-32 of candidates -> t32
                v8 = small.tile([128, K], fp32)
                for r in range(4):
                    nc.vector.max(out=v8[:, r * 8:(r + 1) * 8], in_=cands[:])
                    if r < 3:
                        nc.vector.match_replace(
                            out=cands[:], in_to_replace=v8[:, r * 8:(r + 1) * 8],
                            in_values=cands[:], imm_value=NEG_BIG)

                # neg_tprime = -prevfloat(t32) = t32 * -(1 - 2^-24)   (on ACT)
                ntp = small.tile([128, 1], fp32)
                nc.scalar.activation(out=ntp[:], in_=v8[:, K - 1:K], func=Copy,
                                     scale=-ONE_MINUS_EPS)

                # maskpm = Sign(A - prevfloat(t32)) in {+1,-1} bf16, quarters;
                # accum quarters summed -> detector (== 2K - N iff exact)
                QW = N // 4           # quarter width (2048)
                QCH = NCH // 4        # chunks per quarter (16)
                ps = psum_pool.tile([128, D], fp32)
                acc4 = small.tile([128, 4], fp32)
                for q in range(4):
                    maskb = mpool.tile([128, QW], bf16, tag=f"mb{q}")
                    nc.scalar.activation(
                        out=maskb[:], in_=atq[q][:], func=Sign,
                        bias=ntp[:, 0:1], scale=1.0,
                        accum_out=acc4[:, q:q + 1])

                    # transpose quarter: maskT[p, c, i] = maskpm_q[i, c*128+p]
                    maskT = mtpool.tile([128, QCH * 128], bf16, tag=f"mt{q}")
                    nc.gpsimd.dma_gather(
                        out_ap=maskT[:].rearrange("p (c i) -> p c i", i=128),
                        in_ap=maskb[:], idxs_ap=tidx[:],
                        num_idxs=128, num_idxs_reg=128, elem_size=QW,
                        transpose=True,
                        sbuf_tokens_per_rank=128, sbuf_free_dim_per_rank=QW * 2)
                    mT = maskT[:].rearrange("p (c i) -> p c i", i=128)

                    for c in range(QCH):
                        nc.tensor.matmul(
                            ps[:], lhsT=mT[:, c, :], rhs=xv[:, q * QCH + c, :],
                            start=(q == 0 and c == 0),
                            stop=(q == 3 and c == QCH - 1))

                nc.vector.reduce_sum(cnt_all[:, b:b + 1], acc4[:],
                                     axis=mybir.AxisListType.X)

                # out = (alpha/2) * psum + Xmod
                ot = small.tile([128, D], fp32)
                nc.vector.scalar_tensor_tensor(
                    out=ot[:], in0=ps[:], scalar=alpha_h[:, 0:1],
                    in1=xmod[:, b * D:(b + 1) * D], op0=mult, op1=add)
                nc.sync.dma_start(out=out_ext[b * BATCH:(b + 1) * BATCH, :], in_=ot[:])

            if loop_reps == 1:
                for b in range(NBATCH):
                    batch_body(b)
            else:
                with tc.For_i(0, loop_reps, 1):
                    load_at(0)
                    load_at(1)
                    for b in range(NBATCH):
                        batch_body(b)

            # counts: cnt_all[p, b] -> count[b*128 + p]
            nc.sync.dma_start(
                out=cnt_ext.rearrange("(b p) one -> p (b one)", p=128),
                in_=cnt_all[:],
            )
    nc.compile()
    return nc


def _tidx():
    t = np.zeros((16, 8), np.int16)
    for i in range(128):
        t[i % 16, i // 16] = i
    return np.tile(t, (8, 1))


def make_in_maps(A, X, alpha):
    import ml_dtypes
    Xb = X.astype(ml_dtypes.bfloat16)
    alpha_h = np.full((128, 1), np.float32(alpha) / np.float32(2.0), np.float32)
    tidx = _tidx()
    return [{
        "a": A[c * RPC:(c + 1) * RPC],
        "xb": Xb,
        "xself": X[c * RPC:(c + 1) * RPC],
        "alpha_h": alpha_h,
        "tidx": tidx,
    } for c in range(NCORES)]


def kernel(**inputs):
    global last_results
    from concourse.bass_utils import run_bass_kernel_spmd

    A = np.ascontiguousarray(np.asarray(inputs["A"], dtype=np.float32))
    X = np.ascontiguousarray(np.asarray(inputs["X"], dtype=np.float32))
    alpha = np.float32(np.asarray(inputs["alpha"]))
    k = int(np.asarray(inputs["k"]))
    assert A.shape == (N, N) and X.shape == (N, D)
    if k != K:
        # Safety net for an unexpected k: exact host computation.
        idx = np.argsort(-A, axis=1, kind="stable")[:, :k]
        agg = X[idx].sum(axis=1, dtype=np.float32)
        return (X + alpha * agg).astype(np.float32)

    nc = _build_cached()
    in_maps = make_in_maps(A, X, alpha)

    trace = bool(int(os.environ.get("TOPK_TRACE", "0")))
    res = run_bass_kernel_spmd(nc, in_maps, core_ids=list(range(NCORES)),
                               trace=trace)
    last_results = res

    out = np.concatenate([r["out"] for r in res.results], axis=0)
    accs = np.concatenate([r["count"] for r in res.results], axis=0)[:, 0]

    # Host fallback for rows where the device selection is not exactly top-k
    # (boundary value ties, segment overflow, Sign hitting exact zero).
    bad = np.flatnonzero(accs != np.float32(2 * K - N))
    for r in bad:
        order = np.argsort(-A[r], kind="stable")[:K]
        out[r] = X[r] + alpha * X[order].sum(axis=0, dtype=np.float32)

    return out.astype(np.float32, copy=False)



# revision 6
# speedup vs baseline: 1.0080x; 1.0080x over previous
"""Trainium2 Bass kernel for topk_masking:  out = X + alpha * (top32_mask(A) @ X).

Row-parallel across 8 NeuronCores (A sharded [1024, 8192] per core, X
replicated).  Per 128-row batch on each core, split into quarter-row tiles
(1 MB loads) for fine-grained pipelining:
  * VectorE: per-segment max8 over each quarter -> candidate top-8s, then 4
    rounds of max+match_replace over the candidates -> top-32 values;
    t32 = 32nd largest.  Exact unless >8 of a row's top-32 fall in one
    segment (9 rows for this data; detected and host-fixed).
  * ScalarE: maskpm = Sign(A - prevfloat(t32)) in fp8e4 (+1 selected, -1 not),
    with fused accumulation as an exactness detector (catches segment
    overflow, boundary-value ties, Sign==0).
  * GPSIMD dma_gather(transpose): the fp8 mask viewed as int16 PAIRS is
    transposed (the gather transposes at 16-bit granularity), so partition p
    of the result holds columns (2p, 2p+1) interleaved per byte.  Half the
    SBUF->SBUF DMA traffic of a bf16 mask.
  * TensorE: psum += maskT(fp8, strided byte view) @ Xpar (bf16), where X is
    host-permuted into parity-interleaved chunks so each byte-parity matmul
    contracts against the matching X rows.  mask01 @ X =
    (maskpm @ X + colsum(X)) / 2, so out = [X_self + (a/2) colsum]
    + (a/2) psum, with colsum from an all-ones matmul once per core.
  * VectorE: out = (a/2) * psum + Xmod; DMA out.
Host: rows whose detector count != 32 are recomputed exactly (~11 rows).
"""

import os
import numpy as np

N = 8192
D = 256
K = 32
NCORES = 8
RPC = N // NCORES          # rows per core = 1024
BATCH = 128
NBATCH = RPC // BATCH      # 8
SEG = int(os.environ.get("TOPK_SEG", "512"))
NCH = N // 128             # 64 contraction chunks (of 128)
NPCH = N // 256            # 32 pair-chunks (of 256)
NEG_BIG = -1e30
ONE_MINUS_EPS = float(np.float32(1.0) - np.float32(2.0 ** -24))

last_results = None
_nc_cache = {}


def _build_cached(loop_reps=1, seg=None):
    key = (loop_reps, seg or SEG)
    if key not in _nc_cache:
        _nc_cache[key] = _build(loop_reps, seg)
    return _nc_cache[key]


def _build(loop_reps=1, seg=None):
    import concourse.bacc as bacc
    import concourse.mybir as mybir
    from concourse.tile import TileContext
    from concourse import library_config

    seg = seg or SEG
    nseg = N // seg            # segments per full row
    fp32 = mybir.dt.float32
    bf16 = mybir.dt.bfloat16
    fp8 = mybir.dt.float8e4
    i16 = mybir.dt.int16
    add = mybir.AluOpType.add
    mult = mybir.AluOpType.mult
    Sign = mybir.ActivationFunctionType.Sign
    Copy = mybir.ActivationFunctionType.Copy

    nc = bacc.Bacc("TRN2", debug=False)
    a_in = nc.declare_dram_parameter("a", [RPC, N], fp32, isOutput=False)
    xp_in = nc.declare_dram_parameter("xpar", [128, NCH * D], bf16, isOutput=False)
    xs_in = nc.declare_dram_parameter("xself", [RPC, D], fp32, isOutput=False)
    al_in = nc.declare_dram_parameter("alpha_h", [128, 1], fp32, isOutput=False)
    ti_in = nc.declare_dram_parameter("tidx", [128, 8], mybir.dt.int16, isOutput=False)
    out_ext = nc.declare_dram_parameter("out", [RPC, D], fp32, isOutput=True)
    cnt_ext = nc.declare_dram_parameter("count", [RPC, 1], fp32, isOutput=True)

    abufs = int(os.environ.get("TOPK_ABUFS", "3"))

    with TileContext(nc) as tc:
        with (
            tc.tile_pool(name="persist", bufs=1) as persist,
            tc.tile_pool(name="apool", bufs=abufs) as apool,
            tc.tile_pool(name="mpool", bufs=2) as mpool,
            tc.tile_pool(name="mtpool", bufs=int(os.environ.get("TOPK_MTBUFS", "2"))) as mtpool,
            tc.tile_pool(name="small", bufs=2) as small,
            tc.tile_pool(name="opool", bufs=3) as opool,
            tc.tile_pool(name="psum", bufs=3, space="PSUM") as psum_pool,
            tc.tile_pool(name="psumc", bufs=1, space="PSUM") as psumc_pool,
        ):
            nc.gpsimd.load_library(library_config.mlp)

            at_tiles = {}

            QW4 = N // 4

            def load_at(b):
                qs = []
                for q in range(4):
                    atq = apool.tile([128, QW4], fp32, tag=f"at{q}")
                    nc.sync.dma_start(
                        out=atq[:],
                        in_=a_in[b * BATCH:(b + 1) * BATCH, q * QW4:(q + 1) * QW4])
                    qs.append(atq)
                at_tiles[b] = qs

            if loop_reps == 1:
                load_at(0)
                load_at(1)

            tidx = persist.tile([128, 8], mybir.dt.int16)
            nc.scalar.dma_start(out=tidx[:], in_=ti_in[:])

            # X resident in bf16, parity-interleaved chunk-major:
            # xpar[p, g*D + d] = X[(g>>1)*256 + 2p + (g&1), d]
            xpar = persist.tile([128, NCH * D], bf16)
            nc.scalar.dma_start(out=xpar[:], in_=xp_in[:])
            alpha_h = persist.tile([128, 1], fp32)
            nc.scalar.dma_start(out=alpha_h[:], in_=al_in[:])
            cnt_all = persist.tile([128, NBATCH], fp32)

            xv = xpar[:].rearrange("p (g d) -> p g d", d=D)

            # colsum(X) broadcast to 128 rows via all-ones matmul
            ones_sb = persist.tile([128, 128], bf16)
            nc.vector.memset(ones_sb[:], 1.0)
            ps_cs = psumc_pool.tile([128, D], fp32)
            for g in range(NCH):
                nc.tensor.matmul(ps_cs[:], lhsT=ones_sb[:], rhs=xv[:, g, :],
                                 start=(g == 0), stop=(g == NCH - 1))

            # Xmod = X_self + (alpha/2) * colsum   (per 128-row slice)
            xmod = persist.tile([128, NBATCH * D], fp32)
            for b in range(NBATCH):
                xs = small.tile([128, D], fp32)
                nc.scalar.dma_start(out=xs[:], in_=xs_in[b * BATCH:(b + 1) * BATCH, :])
                nc.vector.scalar_tensor_tensor(
                    out=xmod[:, b * D:(b + 1) * D], in0=ps_cs[:],
                    scalar=alpha_h[:, 0:1], in1=xs[:], op0=mult, op1=add)

            # Out-stage runs with a 2-batch lag so its psum-wait is already
            # satisfied when it reaches the DVE queue head (no head-of-line
            # blocking of the next batch's scan).
            pending = []

            def flush_out():
                bp, psp, accp = pending.pop(0)
                nc.vector.reduce_sum(cnt_all[:, bp:bp + 1], accp[:],
                                     axis=mybir.AxisListType.X)
                ot = opool.tile([128, D], fp32)
                nc.vector.scalar_tensor_tensor(
                    out=ot[:], in0=psp[:], scalar=alpha_h[:, 0:1],
                    in1=xmod[:, bp * D:(bp + 1) * D], op0=mult, op1=add)
                nc.sync.dma_start(out=out_ext[bp * BATCH:(bp + 1) * BATCH, :],
                                  in_=ot[:])

            def batch_body(b):
                if b + 2 < NBATCH:
                    load_at(b + 2)
                atq = at_tiles.pop(b)
                if len(pending) >= 2:
                    flush_out()

                # per-segment top-8 candidates, quarter by quarter
                qseg = QW4 // seg
                cands = small.tile([128, nseg * 8], fp32)
                for q in range(4):
                    for s in range(qseg):
                        g = q * qseg + s
                        nc.vector.max(out=cands[:, g * 8:(g + 1) * 8],
                                      in_=atq[q][:, s * seg:(s + 1) * seg])

                # top-32 of candidates -> t32
                v8 = small.tile([128, K], fp32)
                for r in range(4):
                    nc.vector.max(out=v8[:, r * 8:(r + 1) * 8], in_=cands[:])
                    if r < 3:
                        nc.vector.match_replace(
                            out=cands[:], in_to_replace=v8[:, r * 8:(r + 1) * 8],
                            in_values=cands[:], imm_value=NEG_BIG)

                # neg_tprime = -prevfloat(t32) = t32 * -(1 - 2^-24)   (on ACT)
                ntp = small.tile([128, 1], fp32)
                nc.scalar.activation(out=ntp[:], in_=v8[:, K - 1:K], func=Copy,
                                     scale=-ONE_MINUS_EPS)

                # maskpm = Sign(A - prevfloat(t32)) in {+1,-1} fp8e4, quarters;
                # accum quarters summed -> detector (== 2K - N iff exact)
                QW = N // 4           # quarter width (2048)
                QPCH = QW // 256      # pair-chunks per quarter (8)
                ps = psum_pool.tile([128, D], fp32)
                acc4 = opool.tile([128, 4], fp32)
                for q in range(4):
                    maskb = mpool.tile([128, QW], fp8, tag=f"mb{q}")
                    nc.scalar.activation(
                        out=maskb[:], in_=atq[q][:], func=Sign,
                        bias=ntp[:, 0:1], scale=1.0,
                        accum_out=acc4[:, q:q + 1])

                    # transpose quarter at int16 granularity: partition p of
                    # pair-chunk c holds bytes (mask[r, c*256+2p], [.., +2p+1])
                    maskT = mtpool.tile([128, QPCH * 128], i16, tag=f"mt{q}")
                    nc.gpsimd.dma_gather(
                        out_ap=maskT[:].rearrange("p (c i) -> p c i", i=128),
                        in_ap=maskb[:].bitcast(i16), idxs_ap=tidx[:],
                        num_idxs=128, num_idxs_reg=128, elem_size=QW // 2,
                        transpose=True,
                        sbuf_tokens_per_rank=128, sbuf_free_dim_per_rank=QW)
                    mT = maskT[:].bitcast(fp8).rearrange(
                        "p (c i par) -> p c i par", i=128, par=2)

                    for c in range(QPCH):
                        for par in range(2):
                            g = q * (2 * QPCH) + c * 2 + par
                            nc.tensor.matmul(
                                ps[:], lhsT=mT[:, c, :, par], rhs=xv[:, g, :],
                                start=(g == 0),
                                stop=(g == NCH - 1))

                pending.append((b, ps, acc4))

            if loop_reps == 1:
                for b in range(NBATCH):
                    batch_body(b)
                while pending:
                    flush_out()
            else:
                with tc.For_i(0, loop_reps, 1):
                    load_at(0)
                    load_at(1)
                    for b in range(NBATCH):
                        batch_body(b)
                    while pending:
                        flush_out()

            # counts: cnt_all[p, b] -> count[b*128 + p]
            nc.sync.dma_start(
                out=cnt_ext.rearrange("(b p) one -> p (b one)", p=128),
                in_=cnt_all[:],
            )
    nc.compile()
    return nc


def _tidx():
    t = np.zeros((16, 8), np.int16)
    for i in range(128):
        t[i % 16, i // 16] = i
    return np.tile(t, (8, 1))


def _xpar(X):
    import ml_dtypes
    # xpar[p, g, d] = X[(g>>1)*256 + 2p + (g&1), d]
    Xr = X.reshape(NPCH, 128, 2, D)          # [cc, p, parity, d]
    xp = Xr.transpose(1, 0, 2, 3).reshape(128, NCH * D)
    return np.ascontiguousarray(xp).astype(ml_dtypes.bfloat16)


def make_in_maps(A, X, alpha):
    xpar = _xpar(X)
    alpha_h = np.full((128, 1), np.float32(alpha) / np.float32(2.0), np.float32)
    tidx = _tidx()
    return [{
        "a": A[c * RPC:(c + 1) * RPC],
        "xpar": xpar,
        "xself": X[c * RPC:(c + 1) * RPC],
        "alpha_h": alpha_h,
        "tidx": tidx,
    } for c in range(NCORES)]


def kernel(**inputs):
    global last_results
    from concourse.bass_utils import run_bass_kernel_spmd

    A = np.ascontiguousarray(np.asarray(inputs["A"], dtype=np.float32))
    X = np.ascontiguousarray(np.asarray(inputs["X"], dtype=np.float32))
    alpha = np.float32(np.asarray(inputs["alpha"]))
    k = int(np.asarray(inputs["k"]))
    assert A.shape == (N, N) and X.shape == (N, D)
    if k != K:
        # Safety net for an unexpected k: exact host computation.
        idx = np.argsort(-A, axis=1, kind="stable")[:, :k]
        agg = X[idx].sum(axis=1, dtype=np.float32)
        return (X + alpha * agg).astype(np.float32)

    nc = _build_cached()
    in_maps = make_in_maps(A, X, alpha)

    trace = bool(int(os.environ.get("TOPK_TRACE", "0")))
    res = run_bass_kernel_spmd(nc, in_maps, core_ids=list(range(NCORES)),
                               trace=trace)
    last_results = res

    out = np.concatenate([r["out"] for r in res.results], axis=0)
    accs = np.concatenate([r["count"] for r in res.results], axis=0)[:, 0]

    # Host fallback for rows where the device selection is not exactly top-k
    # (boundary value ties, segment overflow, Sign hitting exact zero).
    bad = np.flatnonzero(accs != np.float32(2 * K - N))
    for r in bad:
        order = np.argsort(-A[r], kind="stable")[:K]
        out[r] = X[r] + alpha * X[order].sum(axis=0, dtype=np.float32)

    return out.astype(np.float32, copy=False)


# revision 10
# speedup vs baseline: 1.0915x; 1.0828x over previous
"""Trainium2 Bass kernel for topk_masking:  out = X + alpha * (top32_mask(A) @ X).

Row-parallel across 8 NeuronCores (A sharded [1024, 8192] per core, X
replicated).  Per 128-row batch on each core:
  * VectorE: per-segment max8 -> candidate top-8s, then 4 rounds of
    max+match_replace -> top-32 values; t32 = 32nd largest.  Exact unless >8
    of a row's top-32 fall in one segment (detected and host-fixed).
  * ScalarE: maskpm = Sign(A - prevfloat(t32)) in fp8e4 (+1 selected, -1 not),
    with fused accumulation as an exactness detector.
  * GPSIMD dma_gather(transpose): fp8 mask pairs viewed as int16 are
    transposed per super-batch of 512 rows (the gather transposes at 16-bit
    granularity), so partition p holds columns (2p, 2p+1) byte-interleaved.
  * TensorE (operands swapped for fast weight loads): X chunk-halves (bf16,
    contiguous) are the stationary operand; the strided fp8 maskT is the
    moving operand at N=512.  psumT[d, r] accumulates (maskpm @ X).T.  The
    psum group is seeded with (2/a)*X_self.T + colsum(X) via an fp32 identity
    matmul (host-precomputed), using mask01 @ X = (maskpm @ X + colsum(X))/2.
  * ScalarE: outT = (a/2) * psumT  (activation Copy with per-partition scale);
    DMA out.  Host transposes outT back to row-major.
Host: rows whose detector count != 32 are recomputed exactly (~11 rows).
"""

import os
import numpy as np

N = 8192
D = 256
K = 32
NCORES = 8
RPC = N // NCORES          # rows per core = 1024
BATCH = 128
NBATCH = RPC // BATCH      # 8
SUPER = 512                # rows per matmul super-batch
NSUPER = RPC // SUPER      # 2
BPS = SUPER // BATCH       # 4 batches per super
SEG = int(os.environ.get("TOPK_SEG", "512"))
NCH = N // 128             # 64 contraction chunks (of 128)
NPCH = N // 256            # 32 pair-chunks (of 256)
NEG_BIG = -1e30
ONE_MINUS_EPS = float(np.float32(1.0) - np.float32(2.0 ** -24))

last_results = None
_nc_cache = {}


def _build_cached(loop_reps=1, seg=None):
    key = (loop_reps, seg or SEG)
    if key not in _nc_cache:
        _nc_cache[key] = _build(loop_reps, seg)
    return _nc_cache[key]


def _build(loop_reps=1, seg=None):
    import concourse.bacc as bacc
    import concourse.mybir as mybir
    from concourse.tile import TileContext
    from concourse import library_config

    seg = seg or SEG
    nseg = N // seg            # segments per full row
    fp32 = mybir.dt.float32
    bf16 = mybir.dt.bfloat16
    fp8 = mybir.dt.float8e4
    i16 = mybir.dt.int16
    Sign = mybir.ActivationFunctionType.Sign
    Copy = mybir.ActivationFunctionType.Copy

    QW = N // 4                # quarter width (2048)

    nc = bacc.Bacc("TRN2", debug=False)
    a_in = nc.declare_dram_parameter("a", [RPC, N], fp32, isOutput=False)
    xp_in = nc.declare_dram_parameter("xpar", [128, NCH * D], bf16, isOutput=False)
    xmT_in = nc.declare_dram_parameter("xmodT", [D, RPC], fp32, isOutput=False)
    id_in = nc.declare_dram_parameter("ident", [128, 128], fp32, isOutput=False)
    al_in = nc.declare_dram_parameter("alpha_h", [128, 1], fp32, isOutput=False)
    ti_in = nc.declare_dram_parameter("tidx", [128, SUPER // 16], mybir.dt.int16,
                                      isOutput=False)
    outT_ext = nc.declare_dram_parameter("outT", [D, RPC], fp32, isOutput=True)
    cnt_ext = nc.declare_dram_parameter("count", [RPC, 1], fp32, isOutput=True)

    abufs = int(os.environ.get("TOPK_ABUFS", "2"))
    mbbufs = int(os.environ.get("TOPK_MBBUFS", "5"))
    mtbufs = int(os.environ.get("TOPK_MTBUFS", "3"))

    with TileContext(nc) as tc:
        with (
            tc.tile_pool(name="persist", bufs=1) as persist,
            tc.tile_pool(name="apool", bufs=abufs) as apool,
            tc.tile_pool(name="mpool", bufs=mbbufs) as mpool,
            tc.tile_pool(name="mtpool", bufs=mtbufs) as mtpool,
            tc.tile_pool(name="small", bufs=2) as small,
            tc.tile_pool(name="accp", bufs=3) as accp,
            tc.tile_pool(name="otp", bufs=3) as otp,
            tc.tile_pool(name="psum", bufs=2, space="PSUM") as psum_pool,
        ):
            nc.gpsimd.load_library(library_config.mlp)

            at_tiles = {}
            QW4 = N // 4

            def load_at(b):
                qs = []
                for q in range(4):
                    atq = apool.tile([128, QW4], fp32, tag=f"at{q}")
                    nc.sync.dma_start(
                        out=atq[:],
                        in_=a_in[b * BATCH:(b + 1) * BATCH, q * QW4:(q + 1) * QW4])
                    qs.append(atq)
                at_tiles[b] = qs

            if loop_reps == 1:
                load_at(0)
                load_at(1)

            tidx = persist.tile([128, SUPER // 16], mybir.dt.int16)
            nc.scalar.dma_start(out=tidx[:], in_=ti_in[:])

            # X resident in bf16, parity-interleaved chunk-major:
            # xpar[p, g*D + d] = X[(g>>1)*256 + 2p + (g&1), d]
            xpar = persist.tile([128, NCH * D], bf16)
            nc.scalar.dma_start(out=xpar[:], in_=xp_in[:])
            # xmodT_pre[d, r] = (2/alpha) * X_self[r, d] + colsum(X)[d]
            xmT = persist.tile([128, 2 * RPC], fp32)
            nc.scalar.dma_start(
                out=xmT[:].rearrange("p (h r) -> p h r", h=2),
                in_=xmT_in.rearrange("(h p) r -> p h r", p=128))
            ident = persist.tile([128, 128], fp32)
            nc.scalar.dma_start(out=ident[:], in_=id_in[:])
            alpha_h = persist.tile([128, 1], fp32)
            nc.scalar.dma_start(out=alpha_h[:], in_=al_in[:])
            cnt_all = persist.tile([128, NBATCH], fp32)

            xv = xpar[:].rearrange("p (g d) -> p g d", d=D)
            acc_tiles = {}

            def scan_batch(b):
                if b + 2 < NBATCH:
                    load_at(b + 2)
                atq = at_tiles.pop(b)

                # per-segment top-8 candidates, quarter by quarter
                qseg = QW4 // seg
                cands = small.tile([128, nseg * 8], fp32)
                for q in range(4):
                    for s in range(qseg):
                        g = q * qseg + s
                        nc.vector.max(out=cands[:, g * 8:(g + 1) * 8],
                                      in_=atq[q][:, s * seg:(s + 1) * seg])

                # top-32 of candidates -> t32
                v8 = small.tile([128, K], fp32)
                for r in range(4):
                    nc.vector.max(out=v8[:, r * 8:(r + 1) * 8], in_=cands[:])
                    if r < 3:
                        nc.vector.match_replace(
                            out=cands[:], in_to_replace=v8[:, r * 8:(r + 1) * 8],
                            in_values=cands[:], imm_value=NEG_BIG)

                # detector readout for batch b-1 (acc ready; no DVE stall)
                if b - 1 in acc_tiles:
                    accp_t = acc_tiles.pop(b - 1)
                    nc.vector.reduce_sum(cnt_all[:, b - 1:b], accp_t[:],
                                         axis=mybir.AxisListType.X)

                # neg_tprime = -prevfloat(t32) = t32 * -(1 - 2^-24)   (on ACT)
                ntp = small.tile([128, 1], fp32)
                nc.scalar.activation(out=ntp[:], in_=v8[:, K - 1:K], func=Copy,
                                     scale=-ONE_MINUS_EPS)
                return atq, ntp

            def body():
                for S in range(NSUPER):
                    # 4 maskb tiles for this super (one per quarter), each
                    # [128, 4 batches * 2048] fp8, written by 4 Sign slices
                    mbs = [mpool.tile([128, BPS * QW], fp8, tag="mb",
                                      name=f"mb{S}_{q}")
                           for q in range(4)]
                    for j in range(BPS):
                        b = S * BPS + j
                        atq, ntp = scan_batch(b)
                        acc4 = accp.tile([128, 4], fp32)
                        acc_tiles[b] = acc4
                        for q in range(4):
                            nc.scalar.activation(
                                out=mbs[q][:, j * QW:(j + 1) * QW],
                                in_=atq[q][:], func=Sign,
                                bias=ntp[:, 0:1], scale=1.0,
                                accum_out=acc4[:, q:q + 1])

                    # psumT halves, seeded with xmodT_pre via identity matmul
                    pss = [psum_pool.tile([128, SUPER], fp32, tag=f"ps{h}",
                                          name=f"ps{S}_{h}")
                           for h in range(2)]
                    for h in range(2):
                        nc.tensor.matmul(
                            pss[h][:], lhsT=ident[:],
                            rhs=xmT[:, h * RPC + S * SUPER:
                                    h * RPC + (S + 1) * SUPER],
                            start=True, stop=False, skip_group_check=True)

                    for q in range(4):
                        maskT = mtpool.tile([128, (QW // 256) * SUPER], i16)
                        nc.gpsimd.dma_gather(
                            out_ap=maskT[:].rearrange("p (c i) -> p c i", i=SUPER),
                            in_ap=mbs[q][:].bitcast(i16), idxs_ap=tidx[:],
                            num_idxs=SUPER, num_idxs_reg=SUPER,
                            elem_size=QW // 2, transpose=True,
                            sbuf_tokens_per_rank=128,
                            sbuf_free_dim_per_rank=QW)
                        mT = maskT[:].bitcast(fp8).rearrange(
                            "p (c i par) -> p c i par", i=SUPER, par=2)

                        for c in range(QW // 256):
                            for par in range(2):
                                g = q * 16 + c * 2 + par
                                for h in range(2):
                                    nc.tensor.matmul(
                                        pss[h][:],
                                        lhsT=xv[:, g, h * 128:(h + 1) * 128],
                                        rhs=mT[:, c, :, par],
                                        start=False,
                                        stop=(g == NCH - 1),
                                        skip_group_check=True)

                    # outT = (alpha/2) * psumT   (ACT copy; out-DMA on ACT q)
                    for h in range(2):
                        otT = otp.tile([128, SUPER], fp32)
                        nc.scalar.activation(out=otT[:], in_=pss[h][:],
                                             func=Copy, scale=alpha_h[:, 0:1])
                        nc.scalar.dma_start(
                            out=outT_ext[h * 128:(h + 1) * 128,
                                         S * SUPER:(S + 1) * SUPER],
                            in_=otT[:])

                # last batch's detector readout
                b = NSUPER * BPS - 1
                accp_t = acc_tiles.pop(b)
                nc.vector.reduce_sum(cnt_all[:, b:b + 1], accp_t[:],
                                     axis=mybir.AxisListType.X)

            if loop_reps == 1:
                body()
            else:
                with tc.For_i(0, loop_reps, 1):
                    load_at(0)
                    load_at(1)
                    body()

            # counts: cnt_all[p, b] -> count[b*128 + p]
            nc.sync.dma_start(
                out=cnt_ext.rearrange("(b p) one -> p (b one)", p=128),
                in_=cnt_all[:],
            )
    nc.compile()
    return nc


def _tidx():
    t = np.zeros((16, SUPER // 16), np.int16)
    for i in range(SUPER):
        t[i % 16, i // 16] = i
    return np.tile(t, (8, 1))


def _xpar(X):
    import ml_dtypes
    # xpar[p, g, d] = X[(g>>1)*256 + 2p + (g&1), d]
    Xr = X.reshape(NPCH, 128, 2, D)          # [cc, p, parity, d]
    xp = Xr.transpose(1, 0, 2, 3).reshape(128, NCH * D)
    return np.ascontiguousarray(xp).astype(ml_dtypes.bfloat16)


def make_in_maps(A, X, alpha):
    xpar = _xpar(X)
    alpha_h = np.full((128, 1), np.float32(alpha) / np.float32(2.0), np.float32)
    tidx = _tidx()
    ident = np.eye(128, dtype=np.float32)
    colsum = X.sum(axis=0, dtype=np.float32)
    inv = np.float32(2.0) / np.float32(alpha)
    maps = []
    for c in range(NCORES):
        xs = X[c * RPC:(c + 1) * RPC]
        xmodT = (inv * xs.T + colsum[:, None]).astype(np.float32)
        maps.append({
            "a": A[c * RPC:(c + 1) * RPC],
            "xpar": xpar,
            "xmodT": np.ascontiguousarray(xmodT),
            "ident": ident,
            "alpha_h": alpha_h,
            "tidx": tidx,
        })
    return maps


def kernel(**inputs):
    global last_results
    from concourse.bass_utils import run_bass_kernel_spmd

    A = np.ascontiguousarray(np.asarray(inputs["A"], dtype=np.float32))
    X = np.ascontiguousarray(np.asarray(inputs["X"], dtype=np.float32))
    alpha = np.float32(np.asarray(inputs["alpha"]))
    k = int(np.asarray(inputs["k"]))
    assert A.shape == (N, N) and X.shape == (N, D)
    if k != K or float(alpha) == 0.0:
        # Safety net for unexpected k / alpha: exact host computation.
        idx = np.argsort(-A, axis=1, kind="stable")[:, :k]
        agg = X[idx].sum(axis=1, dtype=np.float32)
        return (X + alpha * agg).astype(np.float32)

    nc = _build_cached()
    in_maps = make_in_maps(A, X, alpha)

    trace = bool(int(os.environ.get("TOPK_TRACE", "0")))
    res = run_bass_kernel_spmd(nc, in_maps, core_ids=list(range(NCORES)),
                               trace=trace)
    last_results = res

    out = np.concatenate(
        [np.ascontiguousarray(r["outT"].T) for r in res.results], axis=0)
    accs = np.concatenate([r["count"] for r in res.results], axis=0)[:, 0]

    # Host fallback for rows where the device selection is not exactly top-k
    # (boundary value ties, segment overflow, Sign hitting exact zero).
    bad = np.flatnonzero(accs != np.float32(2 * K - N))
    for r in bad:
        order = np.argsort(-A[r], kind="stable")[:K]
        out[r] = X[r] + alpha * X[order].sum(axis=0, dtype=np.float32)

    return out.astype(np.float32, copy=False)


# revision 16
# speedup vs baseline: 1.2845x; 1.1768x over previous
"""Trainium2 Bass kernel for topk_masking:  out = X + alpha * (top32_mask(A) @ X).

Row-parallel across 8 NeuronCores (A sharded [1024, 8192] per core, X
replicated).  Per 128-row batch on each core:
  * VectorE: per-segment max8 -> candidate top-8s, then 4 rounds of
    max+match_replace -> top-32 values; t32 = 32nd largest.  Exact unless >8
    of a row's top-32 fall in one segment (detected and host-fixed).
  * ScalarE: maskpm = Sign(A - prevfloat(t32)) in fp8e4 (+1 selected, -1 not),
    with fused accumulation as an exactness detector.
  * GPSIMD dma_gather(transpose): fp8 mask pairs viewed as int16 are
    transposed per super-batch of 512 rows (the gather transposes at 16-bit
    granularity), so partition p holds columns (2p, 2p+1) byte-interleaved.
  * TensorE (operands swapped for fast weight loads): X chunk-halves (bf16,
    contiguous) are the stationary operand; the strided fp8 maskT is the
    moving operand at N=512.  psumT[d, r] accumulates (maskpm @ X).T.  The
    psum group is seeded with (2/a)*X_self.T + colsum(X) via an fp32 identity
    matmul (host-precomputed), using mask01 @ X = (maskpm @ X + colsum(X))/2.
  * ScalarE: outT = (a/2) * psumT  (activation Copy with per-partition scale);
    DMA out.  Host transposes outT back to row-major.
Host: rows whose detector count != 32 are recomputed exactly (~11 rows).
"""

import os
import numpy as np

N = 8192
D = 256
K = 32
NCORES = 8
RPC = N // NCORES          # rows per core = 1024
BATCH = 128
NBATCH = RPC // BATCH      # 8
SUPER = 512                # rows per matmul super-batch
NSUPER = RPC // SUPER      # 2
BPS = SUPER // BATCH       # 4 batches per super
SEG = int(os.environ.get("TOPK_SEG", "512"))
NCH = N // 128             # 64 contraction chunks (of 128)
NPCH = N // 256            # 32 pair-chunks (of 256)
NEG_BIG = -1e30
ONE_MINUS_EPS = float(np.float32(1.0) - np.float32(2.0 ** -24))

last_results = None
_nc_cache = {}


def _build_cached(loop_reps=1, seg=None):
    key = (loop_reps, seg or SEG)
    if key not in _nc_cache:
        _nc_cache[key] = _build(loop_reps, seg)
    return _nc_cache[key]


def _build(loop_reps=1, seg=None):
    import concourse.bacc as bacc
    import concourse.mybir as mybir
    from concourse.tile import TileContext
    from concourse import library_config

    seg = seg or SEG
    nseg = N // seg            # segments per full row
    fp32 = mybir.dt.float32
    bf16 = mybir.dt.bfloat16
    fp8 = mybir.dt.float8e4
    i16 = mybir.dt.int16
    Sign = mybir.ActivationFunctionType.Sign
    Copy = mybir.ActivationFunctionType.Copy

    QW = N // 4                # quarter width (2048)

    nc = bacc.Bacc("TRN2", debug=False)
    a_in = nc.declare_dram_parameter("a", [RPC, N], fp32, isOutput=False)
    xp_in = nc.declare_dram_parameter("xpar", [128, NCH * D], bf16, isOutput=False)
    al_in = nc.declare_dram_parameter("alpha_h", [128, 1], fp32, isOutput=False)
    ti_in = nc.declare_dram_parameter("tidx", [128, SUPER // 16], mybir.dt.int16,
                                      isOutput=False)
    outT_ext = nc.declare_dram_parameter("outT", [D, RPC], fp32, isOutput=True)
    cnt_ext = nc.declare_dram_parameter("count", [RPC, 1], fp32, isOutput=True)

    abufs = int(os.environ.get("TOPK_ABUFS", "3"))
    mbbufs = int(os.environ.get("TOPK_MBBUFS", "5"))
    mtbufs = int(os.environ.get("TOPK_MTBUFS", "3"))

    with TileContext(nc) as tc:
        with (
            tc.tile_pool(name="persist", bufs=1) as persist,
            tc.tile_pool(name="apool", bufs=abufs) as apool,
            tc.tile_pool(name="mpool", bufs=mbbufs) as mpool,
            tc.tile_pool(name="mtpool", bufs=mtbufs) as mtpool,
            tc.tile_pool(name="small", bufs=2) as small,
            tc.tile_pool(name="accp", bufs=3) as accp,
            tc.tile_pool(name="otp", bufs=3) as otp,
            tc.tile_pool(name="psum", bufs=2, space="PSUM") as psum_pool,
        ):
            nc.gpsimd.load_library(library_config.mlp)

            at_tiles = {}
            QW4 = N // 4

            def load_at(b):
                qs = []
                for q in range(4):
                    atq = apool.tile([128, QW4], fp32, tag=f"at{q}")
                    nc.sync.dma_start(
                        out=atq[:],
                        in_=a_in[b * BATCH:(b + 1) * BATCH, q * QW4:(q + 1) * QW4])
                    qs.append(atq)
                at_tiles[b] = qs

            if loop_reps == 1:
                load_at(0)
                load_at(1)

            tidx = persist.tile([128, SUPER // 16], mybir.dt.int16)
            nc.scalar.dma_start(out=tidx[:], in_=ti_in[:])

            # X resident in bf16, parity-interleaved chunk-major:
            # xpar[p, g*D + d] = X[(g>>1)*256 + 2p + (g&1), d]
            xpar = persist.tile([128, NCH * D], bf16)
            nc.scalar.dma_start(out=xpar[:], in_=xp_in[:])
            alpha_h = persist.tile([128, 1], fp32)
            nc.scalar.dma_start(out=alpha_h[:], in_=al_in[:])
            cnt_all = persist.tile([128, NBATCH], fp32)

            xv = xpar[:].rearrange("p (g d) -> p g d", d=D)
            acc_tiles = {}

            def scan_batch(b):
                if b + 2 < NBATCH:
                    load_at(b + 2)
                atq = at_tiles.pop(b)

                # per-segment top-8 candidates, quarter by quarter
                qseg = QW4 // seg
                cands = small.tile([128, nseg * 8], fp32)
                for q in range(4):
                    for s in range(qseg):
                        g = q * qseg + s
                        nc.vector.max(out=cands[:, g * 8:(g + 1) * 8],
                                      in_=atq[q][:, s * seg:(s + 1) * seg])

                # top-32 of candidates -> t32
                v8 = small.tile([128, K], fp32)
                for r in range(4):
                    nc.vector.max(out=v8[:, r * 8:(r + 1) * 8], in_=cands[:])
                    if r < 3:
                        nc.vector.match_replace(
                            out=cands[:], in_to_replace=v8[:, r * 8:(r + 1) * 8],
                            in_values=cands[:], imm_value=NEG_BIG)

                # detector readout for batch b-1 (acc ready; no DVE stall)
                if b - 1 in acc_tiles:
                    accp_t = acc_tiles.pop(b - 1)
                    nc.vector.reduce_sum(cnt_all[:, b - 1:b], accp_t[:],
                                         axis=mybir.AxisListType.X)

                # neg_tprime = -prevfloat(t32) = t32 * -(1 - 2^-24)   (on ACT)
                ntp = small.tile([128, 1], fp32)
                nc.scalar.activation(out=ntp[:], in_=v8[:, K - 1:K], func=Copy,
                                     scale=-ONE_MINUS_EPS)
                return atq, ntp

            def body():
                for S in range(NSUPER):
                    # 4 maskb tiles for this super (one per quarter), each
                    # [128, 4 batches * 2048] fp8, written by 4 Sign slices
                    mbs = [mpool.tile([128, BPS * QW], fp8, tag="mb",
                                      name=f"mb{S}_{q}")
                           for q in range(4)]
                    for j in range(BPS):
                        b = S * BPS + j
                        atq, ntp = scan_batch(b)
                        acc4 = accp.tile([128, 4], fp32)
                        acc_tiles[b] = acc4
                        for q in range(4):
                            nc.scalar.activation(
                                out=mbs[q][:, j * QW:(j + 1) * QW],
                                in_=atq[q][:], func=Sign,
                                bias=ntp[:, 0:1], scale=1.0,
                                accum_out=acc4[:, q:q + 1])

                    # psumT halves: psT[d, r] accumulates (maskpm @ X).T
                    pss = [psum_pool.tile([128, SUPER], fp32, tag=f"ps{h}",
                                          name=f"ps{S}_{h}")
                           for h in range(2)]

                    for q in range(4):
                        maskT = mtpool.tile([128, (QW // 256) * SUPER], i16)
                        nc.gpsimd.dma_gather(
                            out_ap=maskT[:].rearrange("p (c i) -> p c i", i=SUPER),
                            in_ap=mbs[q][:].bitcast(i16), idxs_ap=tidx[:],
                            num_idxs=SUPER, num_idxs_reg=SUPER,
                            elem_size=QW // 2, transpose=True,
                            sbuf_tokens_per_rank=128,
                            sbuf_free_dim_per_rank=QW)
                        mT = maskT[:].bitcast(fp8).rearrange(
                            "p (c i par) -> p c i par", i=SUPER, par=2)

                        for c in range(QW // 256):
                            for par in range(2):
                                g = q * 16 + c * 2 + par
                                for h in range(2):
                                    nc.tensor.matmul(
                                        pss[h][:],
                                        lhsT=xv[:, g, h * 128:(h + 1) * 128],
                                        rhs=mT[:, c, :, par],
                                        start=(g == 0),
                                        stop=(g == NCH - 1))

                    # outT = (alpha/2) * psumT   (ACT copy; out-DMA on ACT q)
                    for h in range(2):
                        otT = otp.tile([128, SUPER], fp32)
                        nc.scalar.activation(out=otT[:], in_=pss[h][:],
                                             func=Copy, scale=alpha_h[:, 0:1])
                        nc.scalar.dma_start(
                            out=outT_ext[h * 128:(h + 1) * 128,
                                         S * SUPER:(S + 1) * SUPER],
                            in_=otT[:])

                # last batch's detector readout
                b = NSUPER * BPS - 1
                accp_t = acc_tiles.pop(b)
                nc.vector.reduce_sum(cnt_all[:, b:b + 1], accp_t[:],
                                     axis=mybir.AxisListType.X)

            if loop_reps == 1:
                body()
            else:
                with tc.For_i(0, loop_reps, 1):
                    load_at(0)
                    load_at(1)
                    body()

            # counts: cnt_all[p, b] -> count[b*128 + p]
            nc.sync.dma_start(
                out=cnt_ext.rearrange("(b p) one -> p (b one)", p=128),
                in_=cnt_all[:],
            )
    nc.compile()
    return nc


def _tidx():
    t = np.zeros((16, SUPER // 16), np.int16)
    for i in range(SUPER):
        t[i % 16, i // 16] = i
    return np.tile(t, (8, 1))


def _xpar(X):
    import ml_dtypes
    # xpar[p, g, d] = X[(g>>1)*256 + 2p + (g&1), d]
    Xr = X.reshape(NPCH, 128, 2, D)          # [cc, p, parity, d]
    xp = Xr.transpose(1, 0, 2, 3).reshape(128, NCH * D)
    return np.ascontiguousarray(xp).astype(ml_dtypes.bfloat16)


def make_in_maps(A, X, alpha):
    xpar = _xpar(X)
    alpha_h = np.full((128, 1), np.float32(alpha) / np.float32(2.0), np.float32)
    tidx = _tidx()
    return [{
        "a": A[c * RPC:(c + 1) * RPC],
        "xpar": xpar,
        "alpha_h": alpha_h,
        "tidx": tidx,
    } for c in range(NCORES)]


def kernel(**inputs):
    global last_results
    from concourse.bass_utils import run_bass_kernel_spmd

    A = np.ascontiguousarray(np.asarray(inputs["A"], dtype=np.float32))
    X = np.ascontiguousarray(np.asarray(inputs["X"], dtype=np.float32))
    alpha = np.float32(np.asarray(inputs["alpha"]))
    k = int(np.asarray(inputs["k"]))
    assert A.shape == (N, N) and X.shape == (N, D)
    if k != K or float(alpha) == 0.0:
        # Safety net for unexpected k / alpha: exact host computation.
        idx = np.argsort(-A, axis=1, kind="stable")[:, :k]
        agg = X[idx].sum(axis=1, dtype=np.float32)
        return (X + alpha * agg).astype(np.float32)

    nc = _build_cached()
    in_maps = make_in_maps(A, X, alpha)

    trace = bool(int(os.environ.get("TOPK_TRACE", "0")))
    res = run_bass_kernel_spmd(nc, in_maps, core_ids=list(range(NCORES)),
                               trace=trace)
    last_results = res

    # Device returns (alpha/2) * (maskpm @ X).T per core; the host adds
    # X + (alpha/2) * colsum(bf16(X)) -- matching the device's bf16 X so the
    # unselected rows' bf16 rounding errors cancel exactly.
    import ml_dtypes
    Xb = X.astype(ml_dtypes.bfloat16).astype(np.float32)
    cs_term = (np.float32(alpha) / np.float32(2.0)) * Xb.sum(
        axis=0, dtype=np.float32)
    out = np.concatenate(
        [r["outT"].T for r in res.results], axis=0) + (X + cs_term[None, :])
    accs = np.concatenate([r["count"] for r in res.results], axis=0)[:, 0]

    # Host fallback for rows where the device selection is not exactly top-k
    # (boundary value ties, segment overflow, Sign hitting exact zero).
    bad = np.flatnonzero(accs != np.float32(2 * K - N))
    for r in bad:
        order = np.argsort(-A[r], kind="stable")[:K]
        out[r] = X[r] + alpha * X[order].sum(axis=0, dtype=np.float32)

    return out.astype(np.float32, copy=False)
